# revision 37
# baseline (speedup 1.0000x reference)
"""Multi-head causal attention (B=1, S=2048, E=2048, H=16, DH=128) on 8 TRN2
NeuronCores.

Sharding: tensor-parallel over heads; core c owns heads 2c and 2c+1.

v2 pipeline (all-bf16 matmuls; PSUM accumulation fp32):
  S1: stream x^T over two HWDGE queues (sync/scalar) + gpsimd SWDGE,
      chunk 0 split into 4 column quarters so the first matmul starts
      at ~2us; compute Q^T/K^T head 0 (8 PSUM accumulators, kt-major).
  Then per q-group g = 0..3:  Q^T/K^T head 1 for group g, V s-blocks
      4g..4g+3 (both heads), attn(g) -> AllGather fires as early as
      possible.  Group 3 is split into two 256-wide halves with
      separate gathers so the last exposed gather is half-size.
  Tail: output projection column-sharded (core c computes
      y[:, 256c:256(c+1)]); gathered O^T tiles are prefetched with one
      4-D SWDGE DMA per group as each AllGather completes.

attention: S^T = K @ Q^T, exp on ScalarE, block-causal mask as post-exp
multiply, denominators via DVE pair-sum + ones-column matmul (half the
matmul count), normalization via exp(-ln(den)) on ScalarE (same act
table as Exp/Identity), a rank-1 ones matmul broadcast, and a DVE mul.
"""
import os
import sys

if "/opt/trn_rl_repo" not in sys.path:
    sys.path.insert(0, "/opt/trn_rl_repo")

import numpy as np

B, S, E, H = 1, 2048, 2048, 16
DH = E // H          # 128
N_CORES = 8
HPC = H // N_CORES   # heads per core = 2
KT = E // 128        # 16 contraction tiles
QG = 512             # q-group width
NG = S // QG         # 4 q-groups
SBK = S // 128       # 16 s/sk blocks
CSL = E // N_CORES   # 256 output columns per core

_CACHE = {}


def _build(fp_name: str):
    import concourse.bass as bass  # noqa: F401
    import concourse.mybir as mybir
    import concourse.tile as tile
    from concourse import bacc

    F32 = mybir.dt.float32
    F32R = mybir.dt.float32r
    BF16 = mybir.dt.bfloat16
    AF = mybir.ActivationFunctionType

    nc = bacc.Bacc("TRN2", target_bir_lowering=False, debug=False,
                   num_devices=N_CORES)

    xT_t = nc.dram_tensor("xT", [E, S], BF16, kind="ExternalInput")
    wq_t = nc.dram_tensor("wq", [128, KT * HPC * DH], BF16, kind="ExternalInput")
    wk_t = nc.dram_tensor("wk", [128, KT * HPC * DH], BF16, kind="ExternalInput")
    wv_t = nc.dram_tensor("wv", [128, KT * HPC * DH], BF16, kind="ExternalInput")
    bq_t = nc.dram_tensor("bq", [DH, HPC], F32, kind="ExternalInput")
    bk_t = nc.dram_tensor("bk", [DH, HPC], F32, kind="ExternalInput")
    bv_t = nc.dram_tensor("bv", [1, HPC * DH], F32, kind="ExternalInput")
    wo_t = nc.dram_tensor("wo", [128, KT * CSL], BF16, kind="ExternalInput")
    bo_t = nc.dram_tensor("bo", [1, CSL], F32, kind="ExternalInput")
    mask_t = nc.dram_tensor("mask", [4 * 128, QG], BF16, kind="ExternalInput")
    y_t = nc.dram_tensor("y", [S, CSL], F32, kind="ExternalOutput")

    xT_r = xT_t.ap().rearrange("(kt p) s -> kt p s", p=128)
    mask_r = mask_t.ap().rearrange("(jm p) q -> jm p q", p=128)

    scale = 1.0 / float(np.sqrt(DH))

    # attention sub-problems: (g, qoff, W).  Half-splits are a net loss:
    # AllGather cost is fixed-overhead dominated (~25us even at half size).
    SUBS = [(0, 0, QG), (1, 0, QG), (2, 0, QG), (3, 0, QG)]

    with tile.TileContext(nc) as tc:
        with tc.tile_pool(name="const", bufs=1) as constp, \
             tc.tile_pool(name="prod", bufs=1) as prodp, \
             tc.tile_pool(name="dram", bufs=1, space="DRAM") as dramp:
            # head-0 Q/K weights first: they gate the first matmul
            wqk_sb = {}
            for nm_ in ("wq", "wk"):
                for hh in range(HPC):
                    wt = constp.tile([128, KT * DH], BF16,
                                     tag=f"w_{nm_}{hh}", name=f"w_{nm_}{hh}")
                    wqk_sb[(nm_, hh)] = wt
            # halves: the first matmuls gate on the first half only
            HKD = KT * DH // 2
            nc.scalar.dma_start(wqk_sb[("wq", 0)][:, 0:HKD],
                                wq_t.ap()[:, 0:HKD])
            bqs = constp.tile([DH, HPC], F32)
            bks = constp.tile([DH, HPC], F32)
            ones_f32 = constp.tile([128, 128], F32)
            nc.vector.memset(ones_f32[:], 1.0)
            ones128 = constp.tile([128, 128], BF16)
            nc.vector.tensor_copy(ones128[:], ones_f32[:])
            bvs = constp.tile([128, HPC * DH], F32)
            bos = constp.tile([128, CSL], F32)
            masks = constp.tile([128, 4 * QG], BF16)
            wos = constp.tile([128, KT * CSL], BF16)

            # --- products ---
            qkt = prodp.tile([128, HPC * S], BF16)   # Q^T, head hh at cols hh*S
            kkt = prodp.tile([128, HPC * S], BF16)   # K^T
            vt = prodp.tile([128, SBK * HPC * DH], BF16)  # V, s-block sb at sb*256

            # cin/cout layout partition-major [p, h, q] so the og reload
            # reads 2KB-contiguous (h,q) slabs per (partition, core) pair —
            # 1KB descriptors starve the collective's SDMA packets.
            cin = {}
            cout = {}
            for si, (g, qoff, W) in enumerate(SUBS):
                cin[si] = dramp.tile([128, HPC, W], BF16, tag=f"cin{si}",
                                     name=f"cin{si}")
                cout[si] = dramp.tile([N_CORES, 128, HPC, W], BF16,
                                      tag=f"cout{si}", name=f"cout{si}",
                                      addr_space="Shared")
            warm_in = dramp.tile([128, 1024], BF16, tag="warm_in",
                                 name="warm_in")
            warm_out = dramp.tile([N_CORES, 128, 1024], BF16, tag="warm_out",
                                  name="warm_out", addr_space="Shared")

            wv_sb = constp.tile([128, KT * HPC * DH], BF16, tag="wv_sb",
                                name="wv_sb")

            with tc.tile_pool(name="pt", bufs=8) as ptp, \
                 tc.tile_pool(name="pa", bufs=8) as padp, \
                 tc.tile_pool(name="osb", bufs=1) as osbp, \
                 tc.tile_pool(name="rec", bufs=2) as recp, \
                 tc.tile_pool(name="bcs", bufs=2) as bcsp:
                o_sbuf = osbp.tile([128, HPC * S], BF16)
                import contextlib
                xstack = contextlib.ExitStack()
                xtp = xstack.enter_context(tc.tile_pool(name="xt", bufs=1))
                xt = xtp.tile([128, KT * S], BF16)

                # ---- S1: stream x^T, Q^T/K^T head 0, 8 PSUM accs ----
                # chunk 0 in 4 column quarters (one per q-group matmul),
                # remaining chunks round-robin over 4 DMA queues.
                def piece_dma(qeng, piece):
                    qeng.dma_start(
                        xt[:, piece * QG:(piece + 1) * QG],
                        xT_r[0][:, piece * QG:(piece + 1) * QG])

                piece_dma(nc.sync, 0)
                piece_dma(nc.scalar, 1)
                piece_dma(nc.sync, 2)
                nc.scalar.dma_start(wqk_sb[("wk", 0)][:, 0:HKD],
                                    wk_t.ap()[:, 0:HKD])
                piece_dma(nc.scalar, 3)
                nc.scalar.dma_start(bqs[:], bq_t.ap()[:])
                nc.scalar.dma_start(bks[:], bk_t.ap()[:])
                # gpsimd (SWDGE, ~3.6us/chunk) takes the earliest chunks;
                # weights interleave by first-need time
                QSCHED = {1: nc.gpsimd, 2: nc.sync, 3: nc.gpsimd,
                          4: nc.scalar, 5: nc.sync, 6: nc.gpsimd,
                          7: nc.scalar, 8: nc.sync, 9: nc.gpsimd,
                          10: nc.scalar, 11: nc.sync, 12: nc.gpsimd,
                          13: nc.scalar, 14: nc.sync, 15: nc.gpsimd}
                for kt in range(1, KT):
                    QSCHED[kt].dma_start(xt[:, kt * S:(kt + 1) * S],
                                         xT_r[kt])
                    if kt == 9:
                        for nm_, t_ in (("wq", wq_t), ("wk", wk_t)):
                            nc.gpsimd.dma_start(
                                wqk_sb[(nm_, 0)][:, HKD:2 * HKD],
                                t_.ap()[:, HKD:2 * HKD])
                    if kt == 12:
                        for nm_, t_ in (("wq", wq_t), ("wk", wk_t)):
                            nc.gpsimd.dma_start(
                                wqk_sb[(nm_, 1)][:],
                                t_.ap()[:, KT * DH:2 * KT * DH])
                    if kt == 15:
                        nc.gpsimd.dma_start(wv_sb[:], wv_t.ap()[:])
                        nc.gpsimd.dma_start(
                            bvs[:], bv_t.ap().to_broadcast((128, HPC * DH)))
                for jm in range(4):
                    nc.scalar.dma_start(masks[:, jm * QG:(jm + 1) * QG],
                                        mask_r[jm])
                # warm the collective ring (big payload warms ring pacing
                # too) so AG0 doesn't pay first-op setup on the critical
                # chain; emitted late so its trigger doesn't block the
                # gpsimd queue's x-stream chunks
                nc.scalar.dma_start(warm_in[:], wo_t.ap()[:, 0:1024])
                nc.gpsimd.collective_compute(
                    "AllGather", mybir.AluOpType.bypass,
                    replica_groups=[list(range(N_CORES))],
                    ins=[warm_in.opt()], outs=[warm_out.opt()])
                nc.gpsimd.dma_start(wos[:], wo_t.ap()[:])
                nc.gpsimd.dma_start(bos[:], bo_t.ap().to_broadcast((128, CSL)))

                with tc.tile_pool(name="psS1", bufs=1, space="PSUM") as psA:
                    qk_accs = {p: {g: psA.tile([128, QG], F32, tag="qk",
                                               name=f"qk1_{p}{g}", bufs=8)
                                   for g in range(NG)} for p in range(2)}
                    for kt in range(KT):
                        for p, (wn, prod, bias) in enumerate(
                                (("wq", qkt, bqs), ("wk", kkt, bks))):
                            wtile = wqk_sb[(wn, 0)][:, kt * DH:(kt + 1) * DH]
                            for g in range(NG):
                                nc.tensor.matmul(
                                    qk_accs[p][g][:], wtile,
                                    xt[:, kt * S + g * QG:
                                       kt * S + (g + 1) * QG],
                                    start=(kt == 0), stop=(kt == KT - 1))
                    for p, (wn, prod, bias) in enumerate(
                            (("wq", qkt, bqs), ("wk", kkt, bks))):
                        for g in range(NG):
                            nc.scalar.activation(
                                prod[:, g * QG:(g + 1) * QG],
                                qk_accs[p][g][:], AF.Identity,
                                bias=bias[:, 0:1])

                def qk_h1_and_v(g):
                    """Q^T/K^T head 1 for q/k-group g + V s-blocks 4g..4g+3."""
                    with tc.tile_pool(name=f"psP{g}", bufs=1,
                                      space="PSUM") as psA:
                        qk_accs = [psA.tile([128, QG], F32, tag="qk",
                                            name=f"qk2_{p}{g}", bufs=2)
                                   for p in range(2)]
                        v_accs = [psA.tile([128, HPC * DH], F32, tag="v",
                                           name=f"v{g}_{b}", bufs=4)
                                  for b in range(4)]
                        for p, (wn, prod, bias) in enumerate(
                                (("wq", qkt, bqs), ("wk", kkt, bks))):
                            for kt in range(KT):
                                wtile = wqk_sb[(wn, 1)][:, kt * DH:
                                                        (kt + 1) * DH]
                                nc.tensor.matmul(
                                    qk_accs[p][:], wtile,
                                    xt[:, kt * S + g * QG:
                                       kt * S + (g + 1) * QG],
                                    start=(kt == 0), stop=(kt == KT - 1))
                            nc.scalar.activation(
                                prod[:, S + g * QG:S + (g + 1) * QG],
                                qk_accs[p][:], AF.Identity,
                                bias=bias[:, 1:2])
                        for bi in range(4):
                            b = 4 * g + bi
                            for kt in range(KT):
                                nc.tensor.matmul(
                                    v_accs[bi][:],
                                    xt[:, kt * S + b * 128:
                                       kt * S + (b + 1) * 128],
                                    wv_sb[:, kt * HPC * DH:
                                          (kt + 1) * HPC * DH],
                                    start=(kt == 0), stop=(kt == KT - 1))
                            nc.vector.tensor_add(
                                vt[:, b * HPC * DH:(b + 1) * HPC * DH],
                                v_accs[bi][:], bvs[:])

                def attn(si):
                    g, qoff, W = SUBS[si]
                    jmax = 4 * g + (qoff + W) // 128 - 1
                    npairs = (jmax + 1) // 2
                    with tc.tile_pool(name=f"psS{si}", bufs=2,
                                      space="PSUM") as psS, \
                         tc.tile_pool(name=f"psO{si}", bufs=2,
                                      space="PSUM") as psO, \
                         tc.tile_pool(name=f"psN{si}", bufs=2,
                                      space="PSUM") as psN:
                        o_acc = [psO.tile([128, W], F32, tag="o",
                                          name=f"o{hh}") for hh in range(HPC)]
                        # denominator accumulated pre-broadcast: ones matrix
                        # stationary -> every partition row holds the sum
                        s_acc = [psN.tile([128, W], F32, tag="n",
                                          name=f"n{hh}") for hh in range(HPC)]

                        jbase = 4 * g + (qoff // 128)

                        def emit_pv(hh, jp, pt, padd, sq0):
                            for dj in range(2):
                                j = 2 * jp + dj
                                sq = max(0, 128 * (j - jbase))
                                nc.tensor.matmul(
                                    o_acc[hh][:, sq:W],
                                    vt[:, j * HPC * DH + hh * DH:
                                       j * HPC * DH + (hh + 1) * DH],
                                    pt[:, dj * W + sq:(dj + 1) * W],
                                    start=(j == 0), stop=(j == jmax))
                            nc.tensor.matmul(
                                s_acc[hh][:, sq0:W], ones128[:],
                                padd[:, sq0:W],
                                start=(jp == 0), stop=(jp == npairs - 1))

                        pend = []
                        for jp in range(npairs):
                            # causal trim: block j only attends q >= sq(j)
                            sqs = [max(0, 128 * (2 * jp + dj - jbase))
                                   for dj in range(2)]
                            for hh in range(HPC):
                                ps = psS.tile([128, 2 * W], F32, tag="s",
                                              name="ps")
                                for dj in range(2):
                                    j = 2 * jp + dj
                                    nc.tensor.matmul(
                                        ps[:, dj * W + sqs[dj]:
                                           (dj + 1) * W],
                                        kkt[:, hh * S + j * 128:
                                            hh * S + (j + 1) * 128],
                                        qkt[:, hh * S + g * QG + qoff
                                            + sqs[dj]:
                                            hh * S + g * QG + qoff + W],
                                        start=True, stop=True)
                                pt = ptp.tile([128, 2 * W], BF16, tag="p",
                                              name="pt",
                                              padded_shape=[128, 2 * QG])
                                if sqs[1] == 0:
                                    nc.scalar.activation(pt[:], ps[:],
                                                         AF.Exp, scale=scale)
                                else:
                                    for dj in range(2):
                                        nc.scalar.activation(
                                            pt[:, dj * W + sqs[dj]:
                                               (dj + 1) * W],
                                            ps[:, dj * W + sqs[dj]:
                                               (dj + 1) * W],
                                            AF.Exp, scale=scale)
                                if 2 * jp >= jbase:
                                    jms = 2 * jp - jbase
                                    if sqs[1] > sqs[0]:
                                        # the [sq0,sq1) gap of dj=1 is never
                                        # written (stale SBUF can be NaN and
                                        # NaN*0 = NaN): zero it for padd
                                        nc.vector.memset(
                                            pt[:, W + sqs[0]:W + sqs[1]], 0.0)
                                    for dj in range(2):
                                        nc.vector.tensor_mul(
                                            pt[:, dj * W + sqs[dj]:
                                               (dj + 1) * W],
                                            pt[:, dj * W + sqs[dj]:
                                               (dj + 1) * W],
                                            masks[:, (jms + dj) * QG + sqs[dj]:
                                                  (jms + dj) * QG + W])
                                padd = padp.tile([128, W], BF16, tag="pa",
                                                 name="padd",
                                                 padded_shape=[128, QG])
                                nc.vector.tensor_add(padd[:, sqs[0]:W],
                                                     pt[:, sqs[0]:W],
                                                     pt[:, W + sqs[0]:2 * W])
                                pend.append((hh, jp, pt, padd, sqs[0]))
                                while len(pend) > 3:
                                    emit_pv(*pend.pop(0))
                        while pend:
                            emit_pv(*pend.pop(0))

                        for hh in range(HPC):
                            bcs_sb = bcsp.tile([128, W], F32, tag="b",
                                               name="bcs_sb",
                                               padded_shape=[128, QG])
                            with nc.allow_low_precision(
                                    reason="softmax denom recip approx"):
                                nc.vector.reciprocal_approx_fast(
                                    bcs_sb[:], s_acc[hh][:])
                            nc.vector.tensor_mul(
                                o_sbuf[:, hh * S + g * QG + qoff:
                                       hh * S + g * QG + qoff + W],
                                o_acc[hh][:], bcs_sb[:])
                            nc.sync.dma_start(
                                cin[si][:, hh, :],
                                o_sbuf[:, hh * S + g * QG + qoff:
                                       hh * S + g * QG + qoff + W])
                    nc.gpsimd.collective_compute(
                        "AllGather",
                        mybir.AluOpType.bypass,
                        replica_groups=[list(range(N_CORES))],
                        ins=[cin[si].opt()],
                        outs=[cout[si].opt()],
                    )

                for g in range(NG):
                    qk_h1_and_v(g)
                    if g < 3:
                        attn(g)
                # x^T no longer needed: free its 8MB so the 4 og tiles fit
                xstack.close()
                with tc.tile_pool(name="og", bufs=1) as ogp:
                    og = {g: ogp.tile([128, KT * QG], BF16, tag=f"og{g}",
                                      name=f"og{g}") for g in range(NG)}
                    attn(3)

                    # og prefetch: one 4-D SWDGE DMA per gather, triggered
                    # on the gpsimd queue as each AllGather lands.  The last
                    # group is split in half so WO(g3) can start on the
                    # first half while the second streams.
                    for si, (g, qoff, W) in enumerate(SUBS):
                        dst = og[g].rearrange("p (kt q) -> p kt q", q=QG)[
                            :, :, qoff:qoff + W].rearrange(
                                "p (c h) q -> p c h q", c=N_CORES)
                        src = cout[si].rearrange("c p h q -> p c h q")
                        if si == len(SUBS) - 1:
                            nc.gpsimd.dma_start(dst[:, 0:4], src[:, 0:4])
                            nc.gpsimd.dma_start(dst[:, 4:8], src[:, 4:8])
                        else:
                            nc.gpsimd.dma_start(dst, src)

                    # ===== tail: output projection, column-sharded =====
                    with tc.tile_pool(name="yst", bufs=4) as ystp, \
                         tc.tile_pool(name="psY", bufs=4,
                                      space="PSUM") as psY:
                        for g in range(NG):
                            for i in range(4):
                                sb = 4 * g + i
                                acc = psY.tile([128, CSL], F32, tag="y",
                                               name="yacc")
                                for kt in range(KT):
                                    nc.tensor.matmul(
                                        acc[:],
                                        og[g][:, kt * QG + i * 128:
                                              kt * QG + (i + 1) * 128],
                                        wos[:, kt * CSL:(kt + 1) * CSL],
                                        start=(kt == 0), stop=(kt == KT - 1))
                                yst = ystp.tile([128, CSL], F32, tag="ys",
                                                name="yst")
                                nc.vector.tensor_add(yst[:], acc[:], bos[:])
                                qeng = (nc.sync, nc.scalar)[sb % 2]
                                qeng.dma_start(
                                    y_t.ap()[sb * 128:(sb + 1) * 128, :],
                                    yst[:])

    nc.compile()
    return nc


def _tilize(w):
    """[E, cols] -> [128, KT*cols]: k-tile kt at columns kt*cols."""
    cols = w.shape[1]
    return np.ascontiguousarray(
        w.reshape(KT, 128, cols).transpose(1, 0, 2).reshape(128, KT * cols))


def _tilize_hm(w):
    """[E, HPC*DH] -> [128, HPC*KT*DH], head-major then k-tile."""
    return np.ascontiguousarray(
        w.reshape(KT, 128, HPC, DH).transpose(1, 2, 0, 3)
        .reshape(128, HPC * KT * DH))


def _prep_inputs(x, Wq, bq, Wk, bk, Wv, bv, WO, bo):
    import ml_dtypes

    f32 = np.float32
    bf16 = ml_dtypes.bfloat16
    xT = np.ascontiguousarray(np.asarray(x, f32)[0].T).astype(bf16)
    Wq = np.asarray(Wq, f32); Wk = np.asarray(Wk, f32); Wv = np.asarray(Wv, f32)
    bq = np.asarray(bq, f32); bk = np.asarray(bk, f32); bv = np.asarray(bv, f32)
    WO = np.asarray(WO, f32); bo = np.asarray(bo, f32)

    jm = np.arange(4)[:, None, None]
    r = np.arange(128)[None, :, None]
    c = np.arange(QG)[None, None, :]
    mask = (128 * jm + r <= c).astype(bf16).reshape(4 * 128, QG)

    in_maps = []
    for cidx in range(N_CORES):
        h0, h1 = HPC * cidx, HPC * cidx + 1
        in_maps.append({
            "xT": xT,
            "wq": _tilize_hm(np.concatenate([Wq[h0], Wq[h1]], 1)).astype(bf16),
            "wk": _tilize_hm(np.concatenate([Wk[h0], Wk[h1]], 1)).astype(bf16),
            "wv": _tilize(np.concatenate([Wv[h0], Wv[h1]], 1)).astype(bf16),
            "bq": np.ascontiguousarray(np.stack([bq[h0], bq[h1]], 1)),
            "bk": np.ascontiguousarray(np.stack([bk[h0], bk[h1]], 1)),
            "bv": np.concatenate([bv[h0], bv[h1]])[None, :].copy(),
            "wo": _tilize(np.ascontiguousarray(
                WO[:, CSL * cidx:CSL * (cidx + 1)])).astype(bf16),
            "bo": bo[CSL * cidx:CSL * (cidx + 1)][None, :].copy(),
            "mask": mask,
        })
    return in_maps


def kernel(x, Wq, bq, Wk, bk, Wv, bv, WO, bo, trace=False, fp_name="bfloat16"):
    from concourse.bass_utils import run_bass_kernel_spmd

    key = "v2"
    if key not in _CACHE:
        _CACHE[key] = _build(fp_name)
    nc = _CACHE[key]

    in_maps = _prep_inputs(x, Wq, bq, Wk, bk, Wv, bv, WO, bo)
    kwargs = {}
    if trace:
        kwargs["trace"] = True
    res = run_bass_kernel_spmd(nc, in_maps, core_ids=list(range(N_CORES)),
                               **kwargs)
    kernel.last_results = res

    y = np.concatenate([res.results[c]["y"] for c in range(N_CORES)], axis=1)
    return y.reshape(B, S, E).astype(np.float32)


# revision 40
# speedup vs baseline: 1.0491x; 1.0491x over previous
"""Multi-head causal attention (B=1, S=2048, E=2048, H=16, DH=128) on 8 TRN2
NeuronCores.

Sharding: tensor-parallel over heads; core c owns heads 2c and 2c+1.

v2 pipeline (all-bf16 matmuls; PSUM accumulation fp32):
  S1: stream x^T over two HWDGE queues (sync/scalar) + gpsimd SWDGE,
      chunk 0 split into 4 column quarters so the first matmul starts
      at ~2us; compute Q^T/K^T head 0 (8 PSUM accumulators, kt-major).
  Then per q-group g = 0..3:  Q^T/K^T head 1 for group g, V s-blocks
      4g..4g+3 (both heads), attn(g) -> AllGather fires as early as
      possible.  Group 3 is split into two 256-wide halves with
      separate gathers so the last exposed gather is half-size.
  Tail: output projection column-sharded (core c computes
      y[:, 256c:256(c+1)]); gathered O^T tiles are prefetched with one
      4-D SWDGE DMA per group as each AllGather completes.

attention: S^T = K @ Q^T, exp on ScalarE, block-causal mask as post-exp
multiply, denominators via DVE pair-sum + ones-column matmul (half the
matmul count), normalization via exp(-ln(den)) on ScalarE (same act
table as Exp/Identity), a rank-1 ones matmul broadcast, and a DVE mul.
"""
import os
import sys

if "/opt/trn_rl_repo" not in sys.path:
    sys.path.insert(0, "/opt/trn_rl_repo")

import numpy as np

B, S, E, H = 1, 2048, 2048, 16
DH = E // H          # 128
N_CORES = 8
HPC = H // N_CORES   # heads per core = 2
KT = E // 128        # 16 contraction tiles
QG = 512             # q-group width
NG = S // QG         # 4 q-groups
SBK = S // 128       # 16 s/sk blocks
CSL = E // N_CORES   # 256 output columns per core

_CACHE = {}


def _build(fp_name: str):
    import concourse.bass as bass  # noqa: F401
    import concourse.mybir as mybir
    import concourse.tile as tile
    from concourse import bacc

    F32 = mybir.dt.float32
    F32R = mybir.dt.float32r
    BF16 = mybir.dt.bfloat16
    AF = mybir.ActivationFunctionType

    nc = bacc.Bacc("TRN2", target_bir_lowering=False, debug=False,
                   num_devices=N_CORES)

    xT_t = nc.dram_tensor("xT", [E, S], BF16, kind="ExternalInput")
    wq_t = nc.dram_tensor("wq", [128, KT * HPC * DH], BF16, kind="ExternalInput")
    wk_t = nc.dram_tensor("wk", [128, KT * HPC * DH], BF16, kind="ExternalInput")
    wv_t = nc.dram_tensor("wv", [128, KT * HPC * DH], BF16, kind="ExternalInput")
    bq_t = nc.dram_tensor("bq", [DH, HPC], F32, kind="ExternalInput")
    bk_t = nc.dram_tensor("bk", [DH, HPC], F32, kind="ExternalInput")
    bv_t = nc.dram_tensor("bv", [1, HPC * DH], F32, kind="ExternalInput")
    wo_t = nc.dram_tensor("wo", [128, KT * CSL], BF16, kind="ExternalInput")
    bo_t = nc.dram_tensor("bo", [1, CSL], F32, kind="ExternalInput")
    mask_t = nc.dram_tensor("mask", [4 * 128, QG], BF16, kind="ExternalInput")
    y_t = nc.dram_tensor("y", [S, CSL], F32, kind="ExternalOutput")

    xT_r = xT_t.ap().rearrange("(kt p) s -> kt p s", p=128)
    mask_r = mask_t.ap().rearrange("(jm p) q -> jm p q", p=128)

    scale = 1.0 / float(np.sqrt(DH))

    # attention sub-problems: (g, qoff, W).  Half-splits are a net loss:
    # AllGather cost is fixed-overhead dominated (~25us even at half size).
    SUBS = [(0, 0, QG), (1, 0, QG), (2, 0, QG), (3, 0, QG)]

    with tile.TileContext(nc) as tc:
        with tc.tile_pool(name="const", bufs=1) as constp, \
             tc.tile_pool(name="prod", bufs=1) as prodp, \
             tc.tile_pool(name="dram", bufs=1, space="DRAM") as dramp:
            # head-0 Q/K weights first: they gate the first matmul
            wqk_sb = {}
            for nm_ in ("wq", "wk"):
                for hh in range(HPC):
                    wt = constp.tile([128, KT * DH], BF16,
                                     tag=f"w_{nm_}{hh}", name=f"w_{nm_}{hh}")
                    wqk_sb[(nm_, hh)] = wt
            # halves: the first matmuls gate on the first half only
            HKD = KT * DH // 2
            nc.scalar.dma_start(wqk_sb[("wq", 0)][:, 0:HKD],
                                wq_t.ap()[:, 0:HKD])
            bqs = constp.tile([DH, HPC], F32)
            bks = constp.tile([DH, HPC], F32)
            ones_f32 = constp.tile([128, 128], F32)
            nc.vector.memset(ones_f32[:], 1.0)
            ones128 = constp.tile([128, 128], BF16)
            nc.vector.tensor_copy(ones128[:], ones_f32[:])
            bvs = constp.tile([128, HPC * DH], F32)
            bos = constp.tile([128, CSL], F32)
            masks = constp.tile([128, 4 * QG], BF16)
            wos = constp.tile([128, KT * CSL], BF16)

            # --- products ---
            qkt = prodp.tile([128, HPC * S], BF16)   # Q^T, head hh at cols hh*S
            kkt = prodp.tile([128, HPC * S], BF16)   # K^T
            vt = prodp.tile([128, SBK * HPC * DH], BF16)  # V, s-block sb at sb*256

            # cin/cout layout partition-major [p, h, q] so the og reload
            # reads 2KB-contiguous (h,q) slabs per (partition, core) pair —
            # 1KB descriptors starve the collective's SDMA packets.
            cin = {}
            cout = {}
            for si, (g, qoff, W) in enumerate(SUBS):
                cin[si] = dramp.tile([128, HPC, W], BF16, tag=f"cin{si}",
                                     name=f"cin{si}")
                cout[si] = dramp.tile([N_CORES, 128, HPC, W], BF16,
                                      tag=f"cout{si}", name=f"cout{si}",
                                      addr_space="Shared")
            warm_in = dramp.tile([128, 1024], BF16, tag="warm_in",
                                 name="warm_in")
            warm_out = dramp.tile([N_CORES, 128, 1024], BF16, tag="warm_out",
                                  name="warm_out", addr_space="Shared")

            wv_sb = constp.tile([128, KT * HPC * DH], BF16, tag="wv_sb",
                                name="wv_sb")

            with tc.tile_pool(name="pt", bufs=8) as ptp, \
                 tc.tile_pool(name="pa", bufs=8) as padp, \
                 tc.tile_pool(name="osb", bufs=1) as osbp, \
                 tc.tile_pool(name="rec", bufs=2) as recp, \
                 tc.tile_pool(name="bcs", bufs=2) as bcsp:
                o_sbuf = osbp.tile([128, HPC * S], BF16)
                import contextlib
                xstack = contextlib.ExitStack()
                xtp = xstack.enter_context(tc.tile_pool(name="xt", bufs=1))
                xt = xtp.tile([128, KT * S], BF16)

                # ---- S1: stream x^T, Q^T/K^T head 0, 8 PSUM accs ----
                # chunk 0 in 4 column quarters (one per q-group matmul),
                # remaining chunks round-robin over 4 DMA queues.
                def piece_dma(qeng, piece):
                    qeng.dma_start(
                        xt[:, piece * QG:(piece + 1) * QG],
                        xT_r[0][:, piece * QG:(piece + 1) * QG])

                piece_dma(nc.sync, 0)
                piece_dma(nc.scalar, 1)
                piece_dma(nc.sync, 2)
                nc.scalar.dma_start(wqk_sb[("wk", 0)][:, 0:HKD],
                                    wk_t.ap()[:, 0:HKD])
                piece_dma(nc.scalar, 3)
                nc.sync.dma_start(warm_in[:], wo_t.ap()[:, 0:1024])
                nc.scalar.dma_start(bqs[:], bq_t.ap()[:])
                nc.scalar.dma_start(bks[:], bk_t.ap()[:])
                # gpsimd (SWDGE, ~3.6us/chunk) takes the earliest chunks;
                # weights interleave by first-need time
                QSCHED = {1: nc.gpsimd, 2: nc.sync, 3: nc.gpsimd,
                          4: nc.scalar, 5: nc.sync, 6: nc.gpsimd,
                          7: nc.scalar, 8: nc.sync, 9: nc.gpsimd,
                          10: nc.scalar, 11: nc.sync, 12: nc.gpsimd,
                          13: nc.scalar, 14: nc.sync, 15: nc.gpsimd}
                for kt in range(1, KT):
                    QSCHED[kt].dma_start(xt[:, kt * S:(kt + 1) * S],
                                         xT_r[kt])
                    if kt == 3:
                        # warm the collective ring early (big payload warms
                        # ring pacing) so the real AGs start fast; placed
                        # here so the trigger barely blocks gpsimd chunks
                        nc.gpsimd.collective_compute(
                            "AllGather", mybir.AluOpType.bypass,
                            replica_groups=[list(range(N_CORES))],
                            ins=[warm_in.opt()], outs=[warm_out.opt()])
                    if kt == 9:
                        for nm_, t_ in (("wq", wq_t), ("wk", wk_t)):
                            nc.gpsimd.dma_start(
                                wqk_sb[(nm_, 0)][:, HKD:2 * HKD],
                                t_.ap()[:, HKD:2 * HKD])
                    if kt == 12:
                        for nm_, t_ in (("wq", wq_t), ("wk", wk_t)):
                            nc.gpsimd.dma_start(
                                wqk_sb[(nm_, 1)][:],
                                t_.ap()[:, KT * DH:2 * KT * DH])
                    if kt == 15:
                        nc.gpsimd.dma_start(wv_sb[:], wv_t.ap()[:])
                        nc.gpsimd.dma_start(
                            bvs[:], bv_t.ap().to_broadcast((128, HPC * DH)))
                for jm in range(4):
                    nc.scalar.dma_start(masks[:, jm * QG:(jm + 1) * QG],
                                        mask_r[jm])
                nc.gpsimd.dma_start(wos[:], wo_t.ap()[:])
                nc.gpsimd.dma_start(bos[:], bo_t.ap().to_broadcast((128, CSL)))

                with tc.tile_pool(name="psS1", bufs=1, space="PSUM") as psA:
                    qk_accs = {p: {g: psA.tile([128, QG], F32, tag="qk",
                                               name=f"qk1_{p}{g}", bufs=8)
                                   for g in range(NG)} for p in range(2)}
                    for kt in range(KT):
                        for p, (wn, prod, bias) in enumerate(
                                (("wq", qkt, bqs), ("wk", kkt, bks))):
                            wtile = wqk_sb[(wn, 0)][:, kt * DH:(kt + 1) * DH]
                            for g in range(NG):
                                nc.tensor.matmul(
                                    qk_accs[p][g][:], wtile,
                                    xt[:, kt * S + g * QG:
                                       kt * S + (g + 1) * QG],
                                    start=(kt == 0), stop=(kt == KT - 1))
                    for p, (wn, prod, bias) in enumerate(
                            (("wq", qkt, bqs), ("wk", kkt, bks))):
                        for g in range(NG):
                            nc.scalar.activation(
                                prod[:, g * QG:(g + 1) * QG],
                                qk_accs[p][g][:], AF.Identity,
                                bias=bias[:, 0:1])

                def qk_h1_and_v(g):
                    """Q^T/K^T head 1 for q/k-group g + V s-blocks 4g..4g+3."""
                    with tc.tile_pool(name=f"psP{g}", bufs=1,
                                      space="PSUM") as psA:
                        qk_accs = [psA.tile([128, QG], F32, tag="qk",
                                            name=f"qk2_{p}{g}", bufs=2)
                                   for p in range(2)]
                        v_accs = [psA.tile([128, HPC * DH], F32, tag="v",
                                           name=f"v{g}_{b}", bufs=4)
                                  for b in range(4)]
                        for p, (wn, prod, bias) in enumerate(
                                (("wq", qkt, bqs), ("wk", kkt, bks))):
                            for kt in range(KT):
                                wtile = wqk_sb[(wn, 1)][:, kt * DH:
                                                        (kt + 1) * DH]
                                nc.tensor.matmul(
                                    qk_accs[p][:], wtile,
                                    xt[:, kt * S + g * QG:
                                       kt * S + (g + 1) * QG],
                                    start=(kt == 0), stop=(kt == KT - 1))
                            nc.scalar.activation(
                                prod[:, S + g * QG:S + (g + 1) * QG],
                                qk_accs[p][:], AF.Identity,
                                bias=bias[:, 1:2])
                        for bi in range(4):
                            b = 4 * g + bi
                            for kt in range(KT):
                                nc.tensor.matmul(
                                    v_accs[bi][:],
                                    xt[:, kt * S + b * 128:
                                       kt * S + (b + 1) * 128],
                                    wv_sb[:, kt * HPC * DH:
                                          (kt + 1) * HPC * DH],
                                    start=(kt == 0), stop=(kt == KT - 1))
                            nc.vector.tensor_add(
                                vt[:, b * HPC * DH:(b + 1) * HPC * DH],
                                v_accs[bi][:], bvs[:])

                def attn(si):
                    g, qoff, W = SUBS[si]
                    jmax = 4 * g + (qoff + W) // 128 - 1
                    npairs = (jmax + 1) // 2
                    with tc.tile_pool(name=f"psS{si}", bufs=2,
                                      space="PSUM") as psS, \
                         tc.tile_pool(name=f"psO{si}", bufs=2,
                                      space="PSUM") as psO, \
                         tc.tile_pool(name=f"psN{si}", bufs=2,
                                      space="PSUM") as psN:
                        o_acc = [psO.tile([128, W], F32, tag="o",
                                          name=f"o{hh}") for hh in range(HPC)]
                        # denominator accumulated pre-broadcast: ones matrix
                        # stationary -> every partition row holds the sum
                        s_acc = [psN.tile([128, W], F32, tag="n",
                                          name=f"n{hh}") for hh in range(HPC)]

                        jbase = 4 * g + (qoff // 128)

                        def emit_pv(hh, jp, pt, padd, sq0):
                            for dj in range(2):
                                j = 2 * jp + dj
                                sq = max(0, 128 * (j - jbase))
                                nc.tensor.matmul(
                                    o_acc[hh][:, sq:W],
                                    vt[:, j * HPC * DH + hh * DH:
                                       j * HPC * DH + (hh + 1) * DH],
                                    pt[:, dj * W + sq:(dj + 1) * W],
                                    start=(j == 0), stop=(j == jmax))
                            nc.tensor.matmul(
                                s_acc[hh][:, sq0:W], ones128[:],
                                padd[:, sq0:W],
                                start=(jp == 0), stop=(jp == npairs - 1))

                        pend = []
                        for jp in range(npairs):
                            # causal trim: block j only attends q >= sq(j)
                            sqs = [max(0, 128 * (2 * jp + dj - jbase))
                                   for dj in range(2)]
                            for hh in range(HPC):
                                ps = psS.tile([128, 2 * W], F32, tag="s",
                                              name="ps")
                                for dj in range(2):
                                    j = 2 * jp + dj
                                    nc.tensor.matmul(
                                        ps[:, dj * W + sqs[dj]:
                                           (dj + 1) * W],
                                        kkt[:, hh * S + j * 128:
                                            hh * S + (j + 1) * 128],
                                        qkt[:, hh * S + g * QG + qoff
                                            + sqs[dj]:
                                            hh * S + g * QG + qoff + W],
                                        start=True, stop=True)
                                pt = ptp.tile([128, 2 * W], BF16, tag="p",
                                              name="pt",
                                              padded_shape=[128, 2 * QG])
                                if sqs[1] == 0:
                                    nc.scalar.activation(pt[:], ps[:],
                                                         AF.Exp, scale=scale)
                                else:
                                    for dj in range(2):
                                        nc.scalar.activation(
                                            pt[:, dj * W + sqs[dj]:
                                               (dj + 1) * W],
                                            ps[:, dj * W + sqs[dj]:
                                               (dj + 1) * W],
                                            AF.Exp, scale=scale)
                                if 2 * jp >= jbase:
                                    jms = 2 * jp - jbase
                                    if sqs[1] > sqs[0]:
                                        # the [sq0,sq1) gap of dj=1 is never
                                        # written (stale SBUF can be NaN and
                                        # NaN*0 = NaN): zero it for padd
                                        nc.vector.memset(
                                            pt[:, W + sqs[0]:W + sqs[1]], 0.0)
                                    for dj in range(2):
                                        nc.vector.tensor_mul(
                                            pt[:, dj * W + sqs[dj]:
                                               (dj + 1) * W],
                                            pt[:, dj * W + sqs[dj]:
                                               (dj + 1) * W],
                                            masks[:, (jms + dj) * QG + sqs[dj]:
                                                  (jms + dj) * QG + W])
                                padd = padp.tile([128, W], BF16, tag="pa",
                                                 name="padd",
                                                 padded_shape=[128, QG])
                                nc.vector.tensor_add(padd[:, sqs[0]:W],
                                                     pt[:, sqs[0]:W],
                                                     pt[:, W + sqs[0]:2 * W])
                                pend.append((hh, jp, pt, padd, sqs[0]))
                                while len(pend) > 3:
                                    emit_pv(*pend.pop(0))
                        while pend:
                            emit_pv(*pend.pop(0))

                        for hh in range(HPC):
                            bcs_sb = bcsp.tile([128, W], F32, tag="b",
                                               name="bcs_sb",
                                               padded_shape=[128, QG])
                            with nc.allow_low_precision(
                                    reason="softmax denom recip approx"):
                                nc.vector.reciprocal_approx_fast(
                                    bcs_sb[:], s_acc[hh][:])
                            nc.vector.tensor_mul(
                                o_sbuf[:, hh * S + g * QG + qoff:
                                       hh * S + g * QG + qoff + W],
                                o_acc[hh][:], bcs_sb[:])
                            nc.sync.dma_start(
                                cin[si][:, hh, :],
                                o_sbuf[:, hh * S + g * QG + qoff:
                                       hh * S + g * QG + qoff + W])
                    nc.gpsimd.collective_compute(
                        "AllGather",
                        mybir.AluOpType.bypass,
                        replica_groups=[list(range(N_CORES))],
                        ins=[cin[si].opt()],
                        outs=[cout[si].opt()],
                    )

                for g in range(NG):
                    qk_h1_and_v(g)
                    if g < 3:
                        attn(g)
                # x^T no longer needed: free its 8MB so the 4 og tiles fit
                xstack.close()
                with tc.tile_pool(name="og", bufs=1) as ogp:
                    og = {g: ogp.tile([128, KT * QG], BF16, tag=f"og{g}",
                                      name=f"og{g}") for g in range(NG)}
                    attn(3)

                    # og prefetch: one 4-D SWDGE DMA per gather, triggered
                    # on the gpsimd queue as each AllGather lands.  The last
                    # group is split in half so WO(g3) can start on the
                    # first half while the second streams.
                    for si, (g, qoff, W) in enumerate(SUBS):
                        dst = og[g].rearrange("p (kt q) -> p kt q", q=QG)[
                            :, :, qoff:qoff + W].rearrange(
                                "p (c h) q -> p c h q", c=N_CORES)
                        src = cout[si].rearrange("c p h q -> p c h q")
                        if si == len(SUBS) - 1:
                            nc.gpsimd.dma_start(dst[:, 0:4], src[:, 0:4])
                            nc.gpsimd.dma_start(dst[:, 4:8], src[:, 4:8])
                        else:
                            nc.gpsimd.dma_start(dst, src)

                    # ===== tail: output projection, column-sharded =====
                    with tc.tile_pool(name="yst", bufs=4) as ystp, \
                         tc.tile_pool(name="psY", bufs=4,
                                      space="PSUM") as psY:
                        for g in range(NG):
                            for i in range(4):
                                sb = 4 * g + i
                                acc = psY.tile([128, CSL], F32, tag="y",
                                               name="yacc")
                                for kt in range(KT):
                                    nc.tensor.matmul(
                                        acc[:],
                                        og[g][:, kt * QG + i * 128:
                                              kt * QG + (i + 1) * 128],
                                        wos[:, kt * CSL:(kt + 1) * CSL],
                                        start=(kt == 0), stop=(kt == KT - 1))
                                yst = ystp.tile([128, CSL], F32, tag="ys",
                                                name="yst")
                                nc.vector.tensor_add(yst[:], acc[:], bos[:])
                                qeng = (nc.sync, nc.scalar)[sb % 2]
                                qeng.dma_start(
                                    y_t.ap()[sb * 128:(sb + 1) * 128, :],
                                    yst[:])

    nc.compile()
    return nc


def _tilize(w):
    """[E, cols] -> [128, KT*cols]: k-tile kt at columns kt*cols."""
    cols = w.shape[1]
    return np.ascontiguousarray(
        w.reshape(KT, 128, cols).transpose(1, 0, 2).reshape(128, KT * cols))


def _tilize_hm(w):
    """[E, HPC*DH] -> [128, HPC*KT*DH], head-major then k-tile."""
    return np.ascontiguousarray(
        w.reshape(KT, 128, HPC, DH).transpose(1, 2, 0, 3)
        .reshape(128, HPC * KT * DH))


def _prep_inputs(x, Wq, bq, Wk, bk, Wv, bv, WO, bo):
    import ml_dtypes

    f32 = np.float32
    bf16 = ml_dtypes.bfloat16
    xT = np.ascontiguousarray(np.asarray(x, f32)[0].T).astype(bf16)
    Wq = np.asarray(Wq, f32); Wk = np.asarray(Wk, f32); Wv = np.asarray(Wv, f32)
    bq = np.asarray(bq, f32); bk = np.asarray(bk, f32); bv = np.asarray(bv, f32)
    WO = np.asarray(WO, f32); bo = np.asarray(bo, f32)

    jm = np.arange(4)[:, None, None]
    r = np.arange(128)[None, :, None]
    c = np.arange(QG)[None, None, :]
    mask = (128 * jm + r <= c).astype(bf16).reshape(4 * 128, QG)

    in_maps = []
    for cidx in range(N_CORES):
        h0, h1 = HPC * cidx, HPC * cidx + 1
        in_maps.append({
            "xT": xT,
            "wq": _tilize_hm(np.concatenate([Wq[h0], Wq[h1]], 1)).astype(bf16),
            "wk": _tilize_hm(np.concatenate([Wk[h0], Wk[h1]], 1)).astype(bf16),
            "wv": _tilize(np.concatenate([Wv[h0], Wv[h1]], 1)).astype(bf16),
            "bq": np.ascontiguousarray(np.stack([bq[h0], bq[h1]], 1)),
            "bk": np.ascontiguousarray(np.stack([bk[h0], bk[h1]], 1)),
            "bv": np.concatenate([bv[h0], bv[h1]])[None, :].copy(),
            "wo": _tilize(np.ascontiguousarray(
                WO[:, CSL * cidx:CSL * (cidx + 1)])).astype(bf16),
            "bo": bo[CSL * cidx:CSL * (cidx + 1)][None, :].copy(),
            "mask": mask,
        })
    return in_maps


def kernel(x, Wq, bq, Wk, bk, Wv, bv, WO, bo, trace=False, fp_name="bfloat16"):
    from concourse.bass_utils import run_bass_kernel_spmd

    key = "v2"
    if key not in _CACHE:
        _CACHE[key] = _build(fp_name)
    nc = _CACHE[key]

    in_maps = _prep_inputs(x, Wq, bq, Wk, bk, Wv, bv, WO, bo)
    kwargs = {}
    if trace:
        kwargs["trace"] = True
    res = run_bass_kernel_spmd(nc, in_maps, core_ids=list(range(N_CORES)),
                               **kwargs)
    kernel.last_results = res

    y = np.concatenate([res.results[c]["y"] for c in range(N_CORES)], axis=1)
    return y.reshape(B, S, E).astype(np.float32)


# revision 42
# speedup vs baseline: 1.1106x; 1.0586x over previous
"""Multi-head causal attention (B=1, S=2048, E=2048, H=16, DH=128) on 8 TRN2
NeuronCores.

Sharding: tensor-parallel over heads; core c owns heads 2c and 2c+1.

v2 pipeline (all-bf16 matmuls; PSUM accumulation fp32):
  S1: stream x^T over two HWDGE queues (sync/scalar) + gpsimd SWDGE,
      chunk 0 split into 4 column quarters so the first matmul starts
      at ~2us; compute Q^T/K^T head 0 (8 PSUM accumulators, kt-major).
  Then per q-group g = 0..3:  Q^T/K^T head 1 for group g, V s-blocks
      4g..4g+3 (both heads), attn(g) -> AllGather fires as early as
      possible.  Group 3 is split into two 256-wide halves with
      separate gathers so the last exposed gather is half-size.
  Tail: output projection column-sharded (core c computes
      y[:, 256c:256(c+1)]); gathered O^T tiles are prefetched with one
      4-D SWDGE DMA per group as each AllGather completes.

attention: S^T = K @ Q^T, exp on ScalarE, block-causal mask as post-exp
multiply, denominators via DVE pair-sum + ones-column matmul (half the
matmul count), normalization via exp(-ln(den)) on ScalarE (same act
table as Exp/Identity), a rank-1 ones matmul broadcast, and a DVE mul.
"""
import os
import sys

if "/opt/trn_rl_repo" not in sys.path:
    sys.path.insert(0, "/opt/trn_rl_repo")

import numpy as np

B, S, E, H = 1, 2048, 2048, 16
DH = E // H          # 128
N_CORES = 8
HPC = H // N_CORES   # heads per core = 2
KT = E // 128        # 16 contraction tiles
QG = 512             # q-group width
NG = S // QG         # 4 q-groups
SBK = S // 128       # 16 s/sk blocks
CSL = E // N_CORES   # 256 output columns per core

_CACHE = {}


def _build(fp_name: str):
    import concourse.bass as bass  # noqa: F401
    import concourse.mybir as mybir
    import concourse.tile as tile
    from concourse import bacc

    F32 = mybir.dt.float32
    F32R = mybir.dt.float32r
    BF16 = mybir.dt.bfloat16
    AF = mybir.ActivationFunctionType

    nc = bacc.Bacc("TRN2", target_bir_lowering=False, debug=False,
                   num_devices=N_CORES)

    xT_t = nc.dram_tensor("xT", [E, S], BF16, kind="ExternalInput")
    wq_t = nc.dram_tensor("wq", [128, KT * HPC * DH], BF16, kind="ExternalInput")
    wk_t = nc.dram_tensor("wk", [128, KT * HPC * DH], BF16, kind="ExternalInput")
    wv_t = nc.dram_tensor("wv", [128, KT * HPC * DH], BF16, kind="ExternalInput")
    bq_t = nc.dram_tensor("bq", [DH, HPC], F32, kind="ExternalInput")
    bk_t = nc.dram_tensor("bk", [DH, HPC], F32, kind="ExternalInput")
    bv_t = nc.dram_tensor("bv", [1, HPC * DH], F32, kind="ExternalInput")
    wo_t = nc.dram_tensor("wo", [128, KT * CSL], BF16, kind="ExternalInput")
    bo_t = nc.dram_tensor("bo", [1, CSL], F32, kind="ExternalInput")
    mask_t = nc.dram_tensor("mask", [4 * 128, QG], BF16, kind="ExternalInput")
    y_t = nc.dram_tensor("y", [S, CSL], F32, kind="ExternalOutput")

    xT_r = xT_t.ap().rearrange("(kt p) s -> kt p s", p=128)
    mask_r = mask_t.ap().rearrange("(jm p) q -> jm p q", p=128)

    scale = 1.0 / float(np.sqrt(DH))

    # attention sub-problems: (g, qoff, W).  Half-splits are a net loss:
    # AllGather cost is fixed-overhead dominated (~25us even at half size).
    SUBS = [(0, 0, QG), (1, 0, QG), (2, 0, QG), (3, 0, QG)]

    with tile.TileContext(nc) as tc:
        with tc.tile_pool(name="const", bufs=1) as constp, \
             tc.tile_pool(name="prod", bufs=1) as prodp, \
             tc.tile_pool(name="dram", bufs=1, space="DRAM") as dramp:
            # head-0 Q/K weights first: they gate the first matmul
            wqk_sb = {}
            for nm_ in ("wq", "wk"):
                for hh in range(HPC):
                    wt = constp.tile([128, KT * DH], BF16,
                                     tag=f"w_{nm_}{hh}", name=f"w_{nm_}{hh}")
                    wqk_sb[(nm_, hh)] = wt
            # halves: the first matmuls gate on the first half only
            HKD = KT * DH // 2
            nc.scalar.dma_start(wqk_sb[("wq", 0)][:, 0:HKD],
                                wq_t.ap()[:, 0:HKD])
            bqs = constp.tile([DH, HPC], F32)
            bks = constp.tile([DH, HPC], F32)
            ones_f32 = constp.tile([128, 128], F32)
            nc.vector.memset(ones_f32[:], 1.0)
            ones128 = constp.tile([128, 128], BF16)
            nc.vector.tensor_copy(ones128[:], ones_f32[:])
            bvs = constp.tile([128, HPC * DH], F32)
            bos = constp.tile([128, CSL], F32)
            masks = constp.tile([128, 4 * QG], BF16)
            wos = constp.tile([128, KT * CSL], BF16)

            # --- products ---
            qkt = prodp.tile([128, HPC * S], BF16)   # Q^T, head hh at cols hh*S
            kkt = prodp.tile([128, HPC * S], BF16)   # K^T
            vt = prodp.tile([128, SBK * HPC * DH], BF16)  # V, s-block sb at sb*256

            # cin/cout layout partition-major [p, h, q] so the og reload
            # reads 2KB-contiguous (h,q) slabs per (partition, core) pair —
            # 1KB descriptors starve the collective's SDMA packets.
            cin = {}
            cout = {}
            for si, (g, qoff, W) in enumerate(SUBS):
                cin[si] = dramp.tile([128, HPC, W], BF16, tag=f"cin{si}",
                                     name=f"cin{si}")
                cout[si] = dramp.tile([N_CORES, 128, HPC, W], BF16,
                                      tag=f"cout{si}", name=f"cout{si}",
                                      addr_space="Shared")
            warm_in = dramp.tile([128, 1024], BF16, tag="warm_in",
                                 name="warm_in")
            warm_out = dramp.tile([N_CORES, 128, 1024], BF16, tag="warm_out",
                                  name="warm_out", addr_space="Shared")

            wv_sb = constp.tile([128, KT * HPC * DH], BF16, tag="wv_sb",
                                name="wv_sb")

            with tc.tile_pool(name="pt", bufs=8) as ptp, \
                 tc.tile_pool(name="pa", bufs=8) as padp, \
                 tc.tile_pool(name="osb", bufs=1) as osbp, \
                 tc.tile_pool(name="rec", bufs=2) as recp, \
                 tc.tile_pool(name="bcs", bufs=2) as bcsp:
                o_sbuf = osbp.tile([128, HPC * S], BF16)
                import contextlib
                xstack = contextlib.ExitStack()
                xtp = xstack.enter_context(tc.tile_pool(name="xt", bufs=1))
                xt = xtp.tile([128, KT * S], BF16)

                # ---- S1: stream x^T, Q^T/K^T head 0, 8 PSUM accs ----
                # chunk 0 in 4 column quarters (one per q-group matmul),
                # remaining chunks round-robin over 4 DMA queues.
                def piece_dma(qeng, piece):
                    qeng.dma_start(
                        xt[:, piece * QG:(piece + 1) * QG],
                        xT_r[0][:, piece * QG:(piece + 1) * QG])

                piece_dma(nc.sync, 0)
                piece_dma(nc.scalar, 1)
                piece_dma(nc.sync, 2)
                nc.scalar.dma_start(wqk_sb[("wk", 0)][:, 0:HKD],
                                    wk_t.ap()[:, 0:HKD])
                piece_dma(nc.scalar, 3)
                nc.scalar.dma_start(bqs[:], bq_t.ap()[:])
                nc.scalar.dma_start(bks[:], bk_t.ap()[:])
                nc.scalar.dma_start(warm_in[:], wo_t.ap()[:, 0:1024])
                # gpsimd (SWDGE, ~3.6us/chunk) takes the earliest chunks;
                # weights interleave by first-need time
                QSCHED = {1: nc.gpsimd, 2: nc.sync, 3: nc.gpsimd,
                          4: nc.scalar, 5: nc.sync, 6: nc.gpsimd,
                          7: nc.scalar, 8: nc.sync, 9: nc.gpsimd,
                          10: nc.scalar, 11: nc.sync, 12: nc.gpsimd,
                          13: nc.scalar, 14: nc.sync, 15: nc.gpsimd}
                for kt in range(1, KT):
                    QSCHED[kt].dma_start(xt[:, kt * S:(kt + 1) * S],
                                         xT_r[kt])
                    if kt == 3:
                        # warm the collective ring early (big payload warms
                        # ring pacing) so the real AGs start fast; placed
                        # here so the trigger barely blocks gpsimd chunks
                        nc.gpsimd.collective_compute(
                            "AllGather", mybir.AluOpType.bypass,
                            replica_groups=[list(range(N_CORES))],
                            ins=[warm_in.opt()], outs=[warm_out.opt()])
                    if kt == 9:
                        for nm_, t_ in (("wq", wq_t), ("wk", wk_t)):
                            nc.gpsimd.dma_start(
                                wqk_sb[(nm_, 0)][:, HKD:2 * HKD],
                                t_.ap()[:, HKD:2 * HKD])
                    if kt == 12:
                        for nm_, t_ in (("wq", wq_t), ("wk", wk_t)):
                            nc.gpsimd.dma_start(
                                wqk_sb[(nm_, 1)][:],
                                t_.ap()[:, KT * DH:2 * KT * DH])
                    if kt == 15:
                        nc.gpsimd.dma_start(wv_sb[:], wv_t.ap()[:])
                        nc.gpsimd.dma_start(
                            bvs[:], bv_t.ap().to_broadcast((128, HPC * DH)))
                for jm in range(4):
                    nc.scalar.dma_start(masks[:, jm * QG:(jm + 1) * QG],
                                        mask_r[jm])
                nc.gpsimd.dma_start(wos[:], wo_t.ap()[:])
                nc.gpsimd.dma_start(bos[:], bo_t.ap().to_broadcast((128, CSL)))

                with tc.tile_pool(name="psS1", bufs=1, space="PSUM") as psA:
                    qk_accs = {p: {g: psA.tile([128, QG], F32, tag="qk",
                                               name=f"qk1_{p}{g}", bufs=8)
                                   for g in range(NG)} for p in range(2)}
                    for kt in range(KT):
                        for p, (wn, prod, bias) in enumerate(
                                (("wq", qkt, bqs), ("wk", kkt, bks))):
                            wtile = wqk_sb[(wn, 0)][:, kt * DH:(kt + 1) * DH]
                            for g in range(NG):
                                nc.tensor.matmul(
                                    qk_accs[p][g][:], wtile,
                                    xt[:, kt * S + g * QG:
                                       kt * S + (g + 1) * QG],
                                    start=(kt == 0), stop=(kt == KT - 1))
                    for p, (wn, prod, bias) in enumerate(
                            (("wq", qkt, bqs), ("wk", kkt, bks))):
                        for g in range(NG):
                            nc.scalar.activation(
                                prod[:, g * QG:(g + 1) * QG],
                                qk_accs[p][g][:], AF.Identity,
                                bias=bias[:, 0:1])

                def qk_h1_and_v(g):
                    """Q^T/K^T head 1 for q/k-group g + V s-blocks 4g..4g+3."""
                    with tc.tile_pool(name=f"psP{g}", bufs=1,
                                      space="PSUM") as psA:
                        qk_accs = [psA.tile([128, QG], F32, tag="qk",
                                            name=f"qk2_{p}{g}", bufs=2)
                                   for p in range(2)]
                        v_accs = [psA.tile([128, HPC * DH], F32, tag="v",
                                           name=f"v{g}_{b}", bufs=4)
                                  for b in range(4)]
                        for p, (wn, prod, bias) in enumerate(
                                (("wq", qkt, bqs), ("wk", kkt, bks))):
                            for kt in range(KT):
                                wtile = wqk_sb[(wn, 1)][:, kt * DH:
                                                        (kt + 1) * DH]
                                nc.tensor.matmul(
                                    qk_accs[p][:], wtile,
                                    xt[:, kt * S + g * QG:
                                       kt * S + (g + 1) * QG],
                                    start=(kt == 0), stop=(kt == KT - 1))
                            nc.scalar.activation(
                                prod[:, S + g * QG:S + (g + 1) * QG],
                                qk_accs[p][:], AF.Identity,
                                bias=bias[:, 1:2])
                        for bi in range(4):
                            b = 4 * g + bi
                            for kt in range(KT):
                                nc.tensor.matmul(
                                    v_accs[bi][:],
                                    xt[:, kt * S + b * 128:
                                       kt * S + (b + 1) * 128],
                                    wv_sb[:, kt * HPC * DH:
                                          (kt + 1) * HPC * DH],
                                    start=(kt == 0), stop=(kt == KT - 1))
                            nc.vector.tensor_add(
                                vt[:, b * HPC * DH:(b + 1) * HPC * DH],
                                v_accs[bi][:], bvs[:])

                def attn(si):
                    g, qoff, W = SUBS[si]
                    jmax = 4 * g + (qoff + W) // 128 - 1
                    npairs = (jmax + 1) // 2
                    with tc.tile_pool(name=f"psS{si}", bufs=2,
                                      space="PSUM") as psS, \
                         tc.tile_pool(name=f"psO{si}", bufs=2,
                                      space="PSUM") as psO, \
                         tc.tile_pool(name=f"psN{si}", bufs=2,
                                      space="PSUM") as psN:
                        o_acc = [psO.tile([128, W], F32, tag="o",
                                          name=f"o{hh}") for hh in range(HPC)]
                        # denominator accumulated pre-broadcast: ones matrix
                        # stationary -> every partition row holds the sum
                        s_acc = [psN.tile([128, W], F32, tag="n",
                                          name=f"n{hh}") for hh in range(HPC)]

                        jbase = 4 * g + (qoff // 128)

                        def emit_pv(hh, jp, pt, padd, sq0):
                            for dj in range(2):
                                j = 2 * jp + dj
                                sq = max(0, 128 * (j - jbase))
                                nc.tensor.matmul(
                                    o_acc[hh][:, sq:W],
                                    vt[:, j * HPC * DH + hh * DH:
                                       j * HPC * DH + (hh + 1) * DH],
                                    pt[:, dj * W + sq:(dj + 1) * W],
                                    start=(j == 0), stop=(j == jmax))
                            nc.tensor.matmul(
                                s_acc[hh][:, sq0:W], ones128[:],
                                padd[:, sq0:W],
                                start=(jp == 0), stop=(jp == npairs - 1))

                        pend = []
                        for jp in range(npairs):
                            # causal trim: block j only attends q >= sq(j)
                            sqs = [max(0, 128 * (2 * jp + dj - jbase))
                                   for dj in range(2)]
                            for hh in range(HPC):
                                ps = psS.tile([128, 2 * W], F32, tag="s",
                                              name="ps")
                                for dj in range(2):
                                    j = 2 * jp + dj
                                    nc.tensor.matmul(
                                        ps[:, dj * W + sqs[dj]:
                                           (dj + 1) * W],
                                        kkt[:, hh * S + j * 128:
                                            hh * S + (j + 1) * 128],
                                        qkt[:, hh * S + g * QG + qoff
                                            + sqs[dj]:
                                            hh * S + g * QG + qoff + W],
                                        start=True, stop=True)
                                pt = ptp.tile([128, 2 * W], BF16, tag="p",
                                              name="pt",
                                              padded_shape=[128, 2 * QG])
                                if sqs[1] == 0:
                                    nc.scalar.activation(pt[:], ps[:],
                                                         AF.Exp, scale=scale)
                                else:
                                    for dj in range(2):
                                        nc.scalar.activation(
                                            pt[:, dj * W + sqs[dj]:
                                               (dj + 1) * W],
                                            ps[:, dj * W + sqs[dj]:
                                               (dj + 1) * W],
                                            AF.Exp, scale=scale)
                                if 2 * jp >= jbase:
                                    jms = 2 * jp - jbase
                                    if sqs[1] > sqs[0]:
                                        # the [sq0,sq1) gap of dj=1 is never
                                        # written (stale SBUF can be NaN and
                                        # NaN*0 = NaN): zero it for padd
                                        nc.vector.memset(
                                            pt[:, W + sqs[0]:W + sqs[1]], 0.0)
                                    for dj in range(2):
                                        nc.vector.tensor_mul(
                                            pt[:, dj * W + sqs[dj]:
                                               (dj + 1) * W],
                                            pt[:, dj * W + sqs[dj]:
                                               (dj + 1) * W],
                                            masks[:, (jms + dj) * QG + sqs[dj]:
                                                  (jms + dj) * QG + W])
                                padd = padp.tile([128, W], BF16, tag="pa",
                                                 name="padd",
                                                 padded_shape=[128, QG])
                                nc.vector.tensor_add(padd[:, sqs[0]:W],
                                                     pt[:, sqs[0]:W],
                                                     pt[:, W + sqs[0]:2 * W])
                                pend.append((hh, jp, pt, padd, sqs[0]))
                                while len(pend) > 3:
                                    emit_pv(*pend.pop(0))
                        while pend:
                            emit_pv(*pend.pop(0))

                        for hh in range(HPC):
                            bcs_sb = bcsp.tile([128, W], F32, tag="b",
                                               name="bcs_sb",
                                               padded_shape=[128, QG])
                            with nc.allow_low_precision(
                                    reason="softmax denom recip approx"):
                                nc.vector.reciprocal_approx_fast(
                                    bcs_sb[:], s_acc[hh][:])
                            nc.vector.tensor_mul(
                                o_sbuf[:, hh * S + g * QG + qoff:
                                       hh * S + g * QG + qoff + W],
                                o_acc[hh][:], bcs_sb[:])
                            nc.sync.dma_start(
                                cin[si][:, hh, :],
                                o_sbuf[:, hh * S + g * QG + qoff:
                                       hh * S + g * QG + qoff + W])
                    nc.gpsimd.collective_compute(
                        "AllGather",
                        mybir.AluOpType.bypass,
                        replica_groups=[list(range(N_CORES))],
                        ins=[cin[si].opt()],
                        outs=[cout[si].opt()],
                    )

                for g in range(NG):
                    qk_h1_and_v(g)
                    if g < 3:
                        attn(g)
                # x^T no longer needed: free its 8MB so the 4 og tiles fit
                xstack.close()
                with tc.tile_pool(name="og", bufs=1) as ogp:
                    og = {g: ogp.tile([128, KT * QG], BF16, tag=f"og{g}",
                                      name=f"og{g}") for g in range(NG)}
                    attn(3)

                    # og prefetch: one 4-D SWDGE DMA per gather, triggered
                    # on the gpsimd queue as each AllGather lands.  The last
                    # group is split in half so WO(g3) can start on the
                    # first half while the second streams.
                    for si, (g, qoff, W) in enumerate(SUBS):
                        dst = og[g].rearrange("p (kt q) -> p kt q", q=QG)[
                            :, :, qoff:qoff + W].rearrange(
                                "p (c h) q -> p c h q", c=N_CORES)
                        src = cout[si].rearrange("c p h q -> p c h q")
                        if si == len(SUBS) - 1:
                            for c4 in range(0, N_CORES, 2):
                                nc.gpsimd.dma_start(dst[:, c4:c4 + 2],
                                                    src[:, c4:c4 + 2])
                        else:
                            nc.gpsimd.dma_start(dst, src)

                    # ===== tail: output projection, column-sharded =====
                    with tc.tile_pool(name="yst", bufs=4) as ystp, \
                         tc.tile_pool(name="psY", bufs=4,
                                      space="PSUM") as psY:
                        for g in range(NG):
                            for i in range(4):
                                sb = 4 * g + i
                                acc = psY.tile([128, CSL], F32, tag="y",
                                               name="yacc")
                                for kt in range(KT):
                                    nc.tensor.matmul(
                                        acc[:],
                                        og[g][:, kt * QG + i * 128:
                                              kt * QG + (i + 1) * 128],
                                        wos[:, kt * CSL:(kt + 1) * CSL],
                                        start=(kt == 0), stop=(kt == KT - 1))
                                yst = ystp.tile([128, CSL], F32, tag="ys",
                                                name="yst")
                                nc.vector.tensor_add(yst[:], acc[:], bos[:])
                                qeng = (nc.sync, nc.scalar)[sb % 2]
                                qeng.dma_start(
                                    y_t.ap()[sb * 128:(sb + 1) * 128, :],
                                    yst[:])

    nc.compile()
    return nc


def _tilize(w):
    """[E, cols] -> [128, KT*cols]: k-tile kt at columns kt*cols."""
    cols = w.shape[1]
    return np.ascontiguousarray(
        w.reshape(KT, 128, cols).transpose(1, 0, 2).reshape(128, KT * cols))


def _tilize_hm(w):
    """[E, HPC*DH] -> [128, HPC*KT*DH], head-major then k-tile."""
    return np.ascontiguousarray(
        w.reshape(KT, 128, HPC, DH).transpose(1, 2, 0, 3)
        .reshape(128, HPC * KT * DH))


def _prep_inputs(x, Wq, bq, Wk, bk, Wv, bv, WO, bo):
    import ml_dtypes

    f32 = np.float32
    bf16 = ml_dtypes.bfloat16
    xT = np.ascontiguousarray(np.asarray(x, f32)[0].T).astype(bf16)
    Wq = np.asarray(Wq, f32); Wk = np.asarray(Wk, f32); Wv = np.asarray(Wv, f32)
    bq = np.asarray(bq, f32); bk = np.asarray(bk, f32); bv = np.asarray(bv, f32)
    WO = np.asarray(WO, f32); bo = np.asarray(bo, f32)

    jm = np.arange(4)[:, None, None]
    r = np.arange(128)[None, :, None]
    c = np.arange(QG)[None, None, :]
    mask = (128 * jm + r <= c).astype(bf16).reshape(4 * 128, QG)

    in_maps = []
    for cidx in range(N_CORES):
        h0, h1 = HPC * cidx, HPC * cidx + 1
        in_maps.append({
            "xT": xT,
            "wq": _tilize_hm(np.concatenate([Wq[h0], Wq[h1]], 1)).astype(bf16),
            "wk": _tilize_hm(np.concatenate([Wk[h0], Wk[h1]], 1)).astype(bf16),
            "wv": _tilize(np.concatenate([Wv[h0], Wv[h1]], 1)).astype(bf16),
            "bq": np.ascontiguousarray(np.stack([bq[h0], bq[h1]], 1)),
            "bk": np.ascontiguousarray(np.stack([bk[h0], bk[h1]], 1)),
            "bv": np.concatenate([bv[h0], bv[h1]])[None, :].copy(),
            "wo": _tilize(np.ascontiguousarray(
                WO[:, CSL * cidx:CSL * (cidx + 1)])).astype(bf16),
            "bo": bo[CSL * cidx:CSL * (cidx + 1)][None, :].copy(),
            "mask": mask,
        })
    return in_maps


def kernel(x, Wq, bq, Wk, bk, Wv, bv, WO, bo, trace=False, fp_name="bfloat16"):
    from concourse.bass_utils import run_bass_kernel_spmd

    key = "v2"
    if key not in _CACHE:
        _CACHE[key] = _build(fp_name)
    nc = _CACHE[key]

    in_maps = _prep_inputs(x, Wq, bq, Wk, bk, Wv, bv, WO, bo)
    kwargs = {}
    if trace:
        kwargs["trace"] = True
    res = run_bass_kernel_spmd(nc, in_maps, core_ids=list(range(N_CORES)),
                               **kwargs)
    kernel.last_results = res

    y = np.concatenate([res.results[c]["y"] for c in range(N_CORES)], axis=1)
    return y.reshape(B, S, E).astype(np.float32)
